# revision 39
# baseline (speedup 1.0000x reference)
"""Trainium2 Bass kernel for DenseConv2d via 1D Winograd F(2,3).

Conv2d: input (32,128,56,56) f32, weight (256,128,3,3) f32, bias (256,) f32,
stride 1, pad 1, dilation 1 -> output (32,256,56,56) f32.

Strategy: data-parallel over batch across 8 NeuronCores (4 images per core).
Per core, the conv is computed with Winograd F(2,3) along the x axis only:
for each x-tile tx (28 per row) the 4 transformed planes
  V0 = d0-d2, V1 = d1+d2, V2 = d2-d1, V3 = d1-d3   (dk = xpad[2tx+k])
are built on-chip by the vector engine from 4 pre-shifted even/odd input
planes (host-prepped so every operand is step-1 and 4B-aligned -> DVE 2x).
The y-axis stays direct: M_j = sum_ky U_j[ky]^T V_j[y+ky], so each of the
4 M-planes is a 3-matmul PSUM accumulation in bf16 (1 col/cycle + FWL).
This cuts PE columns 1.5x vs direct conv.  Outputs
  Y0 = M0+M1+M2+b  (even x),  Y1 = M1-M2-M3+b  (odd x)
drain from PSUM through three rotating engine paths (DVE fused
scalar_tensor_tensor; ACT strided copy + DVE adds; ACT copy + GPSIMD adds)
so no single engine bottlenecks.  Outputs ship as bf16; the host interleaves
even/odd columns and upcasts to f32 (layout-only).
"""

import sys

if "/opt/trn_rl_repo" not in sys.path:
    sys.path.insert(0, "/opt/trn_rl_repo")

import numpy as np

N_CORES = 8
N, CI, H, W = 32, 128, 56, 56
CO, KH, KW = 256, 3, 3
NP_CORE = N // N_CORES          # images per core
HP = H + 2                      # padded rows
TX = W // 2                     # x-tiles per row
XW = 32                         # padded plane row width (28 valid + pad)
COT = CO // 128                 # out-channel tiles of 128
RBW = 14                        # output rows per chunk
NCH = H // RBW                  # chunks per (img, cot)
FD = RBW * TX                   # matmul free dim (392)
N_WARMUP = 5                    # big PE warmup matmuls
N_TINY = 30                     # tiny warmup matmuls

_CACHE = {}


def _build_program():
    import concourse.mybir as mybir
    from concourse import bacc
    from concourse.tile import TileContext

    nc = bacc.Bacc(None, target_bir_lowering=False)

    bf16 = mybir.dt.bfloat16
    f32 = mybir.dt.float32
    ADD = mybir.AluOpType.add
    SUB = mybir.AluOpType.subtract
    COPY = mybir.ActivationFunctionType.Copy
    IDENT = mybir.ActivationFunctionType.Identity

    # xq planes: 0=xe[0:28] (d0), 1=xo[0:28] (d1), 2=xe[1:29] (d2),
    # 3=xo[1:29] (d3); all width-28 valid, padded to 32.  The plane dim
    # sits INSIDE the row dim so row-sliced DMA chunks are contiguous
    # (>=4KB per partition per 16 rows -> saturated DMA descriptors).
    x_d = nc.dram_tensor("xq", [CI, NP_CORE, HP, 4, XW], bf16,
                         kind="ExternalInput")
    w_d = nc.dram_tensor("w", [CI, COT, 4, KH, 128], bf16,
                         kind="ExternalInput")
    # [:, cot, 0] = +bias, [:, cot, 1] = -bias
    b_d = nc.dram_tensor("b2", [128, COT, 2], f32, kind="ExternalInput")
    # y layout: [cot, co_p, img, b(even/odd), y, tx]
    y_d = nc.dram_tensor("y", [COT, 128, NP_CORE, 2, H, TX], bf16,
                         kind="ExternalOutput")

    with TileContext(nc) as tc:
        with (
            tc.tile_pool(name="xin", bufs=1) as xpool,
            tc.tile_pool(name="vpool", bufs=1) as vpool,
            tc.tile_pool(name="wpool", bufs=1) as wpool,
            tc.tile_pool(name="bpool", bufs=1) as bpool,
            tc.tile_pool(name="cpool", bufs=3) as cpool,
            tc.tile_pool(name="tpool", bufs=2) as tpool,
            tc.tile_pool(name="ypool", bufs=3) as ypool,
            tc.tile_pool(name="psum", bufs=2, space="PSUM") as ppool,
        ):
            # --- PE warmup on scratch data (lifts HAM clock gate) ---
            scratch = xpool.tile([CI, FD], bf16, tag="scratch")
            nc.vector.memset(scratch, 0.0)
            wups = ppool.tile([128, 4, 512], f32, tag="m")
            for _ in range(N_WARMUP):
                nc.tensor.matmul(wups[:, 0, 0:FD], scratch[:, 0:128],
                                 scratch, start=True, stop=True)
            for _ in range(N_TINY):
                nc.tensor.matmul(wups[:, 0, 0:64], scratch[:, 0:128],
                                 scratch[:, 0:64], start=True, stop=True)

            # --- DMAs: weights + bias on sync, input planes on scalar ---
            # Input DMAs all ride the scalar ring, img0 first in 4 row-chunks
            # (same-ring transfers serialize, so img0 is prioritized and its
            # first chunk lands fast); weights + bias ride sync.  All y-out
            # DMAs go on sync LATER so they never block the scalar ring's
            # PSUM-drain copies.
            xt = [xpool.tile([CI, HP, 4, XW], bf16, tag=f"x{img}",
                             name=f"x{img}")
                  for img in range(NP_CORE)]
            X0CH = [(0, 16), (16, 30), (30, 44), (44, HP)]
            for r0, r1 in X0CH:
                nc.scalar.dma_start(out=xt[0][:, r0:r1, :, :],
                                    in_=x_d[:, 0, r0:r1, :, :])
            wt = []
            for cot in range(COT):
                wtile = wpool.tile([CI, 4, KH, 128], bf16, tag=f"w{cot}")
                nc.sync.dma_start(out=wtile, in_=w_d[:, cot, :, :, :])
                wt.append(wtile)
            bt = bpool.tile([128, COT, 2], f32)
            nc.sync.dma_start(out=bt, in_=b_d[:, :, :])
            # x1/x2/x3 queue behind x0 on the scalar ring, serialized in
            # image order.  Total early HBM bandwidth is the binding
            # constraint: spreading these across extra rings floods the
            # fabric and starves x0's critical first chunk (measured).
            for img in range(1, NP_CORE):
                nc.scalar.dma_start(out=xt[img], in_=x_d[:, img, :, :, :])

            # only 2 V tiles live at once (current image + prefetched next)
            vt = [vpool.tile([CI, 4, HP, XW], bf16, tag=f"v{img % 2}",
                             name=f"v{img}")
                  for img in range(NP_CORE)]

            def emit_v(img, r0, r1, eng=None):
                eng = eng or nc.vector
                x_ = xt[img]
                v_ = vt[img]
                p = [x_[:, r0:r1, k, 0:28] for k in range(4)]
                o = [v_[:, j, r0:r1, 0:28] for j in range(4)]
                eng.tensor_tensor(out=o[0], in0=p[0], in1=p[2], op=SUB)
                eng.tensor_tensor(out=o[1], in0=p[1], in1=p[2], op=ADD)
                eng.tensor_tensor(out=o[2], in0=p[2], in1=p[1], op=SUB)
                eng.tensor_tensor(out=o[3], in0=p[1], in1=p[3], op=SUB)

            # Per (img, cot): 4 chunks processed as 2 pairs.  Each chunk's
            # 4 M-planes drain via ONE strided ACT copy (frees the PSUM
            # buffer quickly); the Winograd output adds then run over the
            # PAIR (FD=784, amortizing fixed op costs), all on DVE with the
            # bias fused into the first layer:
            #   ye = ((c0 + b) + c1) + c2   = M0+M1+M2+b
            #   yo = ((c1 + b) - c2) - c3   = M1-M2-M3+b
            def emit_chunk_mms(img, cot, ch, c2):
                y0 = ch * RBW
                mt = ppool.tile([128, 4, 512], f32, tag="m")
                for j in range(4):
                    for ky in range(KH):
                        rhs = vt[img][:, j, y0 + ky:y0 + ky + RBW, 0:28]
                        nc.tensor.matmul(
                            mt[:, j, 0:FD], wt[cot][:, j, ky, :], rhs,
                            start=(ky == 0), stop=(ky == KH - 1),
                        )
                nc.scalar.activation(out=c2[:, :, ch % 2, :],
                                     in_=mt[:, :, 0:FD], func=COPY)

            def emit_pair_adds(img, cot, pair, c2, yt):
                bpos = bt[:, cot, 0:1]
                cc = [c2[:, k, :, :] for k in range(4)]   # [128, 2, FD] flat
                ye = yt[:, 0, 2 * pair:2 * pair + 2, :]
                yo = yt[:, 1, 2 * pair:2 * pair + 2, :]
                t0 = tpool.tile([128, 2, FD], bf16, tag="t0")
                nc.vector.scalar_tensor_tensor(
                    out=t0, in0=cc[0], scalar=bpos, in1=cc[1],
                    op0=ADD, op1=ADD)
                t1 = tpool.tile([128, 2, FD], bf16, tag="t1")
                nc.vector.scalar_tensor_tensor(
                    out=t1, in0=cc[1], scalar=bpos, in1=cc[2],
                    op0=ADD, op1=SUB)
                nc.vector.tensor_tensor(out=ye, in0=t0, in1=cc[2], op=ADD)
                nc.vector.tensor_tensor(out=yo, in0=t1, in1=cc[3], op=SUB)

            def emit_compute(img, cot, yt):
                for pair in range(2):
                    # c2 packs both chunks of the pair: [plane, set, FD]
                    c2 = cpool.tile([128, 4, 2, FD], bf16, tag="c")
                    emit_chunk_mms(img, cot, 2 * pair, c2)
                    emit_chunk_mms(img, cot, 2 * pair + 1, c2)
                    emit_pair_adds(img, cot, pair, c2, yt)
                    # ship this half-image's rows while the next pair drains
                    nc.sync.dma_start(
                        out=y_d[cot, :, img, :,
                                pair * 2 * RBW:(pair + 1) * 2 * RBW, :],
                        in_=yt[:, :, 2 * pair:2 * pair + 2, :])

            def emit_compute_last(img, cot, yt):
                # final (img, cot): per-chunk drains + small DMAs so the
                # post-last-matmul tail is as short as possible; the last
                # DMA rides the scalar ring (idle once its copies are done).
                bpos = bt[:, cot, 0:1]
                for ch in range(NCH):
                    c2 = cpool.tile([128, 4, 2, FD], bf16, tag="c")
                    emit_chunk_mms(img, cot, ch, c2)
                    cc = [c2[:, k, ch % 2, :] for k in range(4)]
                    ye = yt[:, 0, ch, :]
                    yo = yt[:, 1, ch, :]
                    t0 = tpool.tile([128, 2, FD], bf16, tag="t0")
                    nc.vector.scalar_tensor_tensor(
                        out=t0[:, 0, :], in0=cc[0], scalar=bpos, in1=cc[1],
                        op0=ADD, op1=ADD)
                    t1 = tpool.tile([128, 2, FD], bf16, tag="t1")
                    nc.vector.scalar_tensor_tensor(
                        out=t1[:, 0, :], in0=cc[1], scalar=bpos, in1=cc[2],
                        op0=ADD, op1=SUB)
                    nc.vector.tensor_tensor(out=ye, in0=t0[:, 0, :],
                                            in1=cc[2], op=ADD)
                    nc.vector.tensor_tensor(out=yo, in0=t1[:, 0, :],
                                            in1=cc[3], op=SUB)
                    q = nc.scalar if ch == NCH - 1 else nc.sync
                    q.dma_start(
                        out=y_d[cot, :, img, :, ch * RBW:(ch + 1) * RBW, :],
                        in_=yt[:, :, ch:ch + 1, :])

            for r0, r1 in X0CH:
                emit_v(0, r0, r1)
            for img in range(NP_CORE):
                for cot in range(COT):
                    yt = ypool.tile([128, 2, NCH, FD], bf16, tag="y")
                    # prefetch next image's V transform between cots
                    if cot == 1 and img + 1 < NP_CORE:
                        emit_v(img + 1, 0, HP)
                    emit_compute(img, cot, yt)

    nc.compile()
    return nc


def prep_in_maps(input, weight, bias):
    """Host-side layout prep -> one in_map per core."""
    import ml_dtypes

    bf16 = ml_dtypes.bfloat16

    # Winograd weight transform (tiny): U_j[ky][ci, co]
    g = weight.transpose(2, 3, 1, 0).astype(np.float32)  # [kh, kw, ci, co]
    U = np.empty((4, KH, CI, CO), dtype=np.float32)
    U[0] = g[:, 0]
    U[1] = (g[:, 0] + g[:, 1] + g[:, 2]) * 0.5
    U[2] = (g[:, 0] - g[:, 1] + g[:, 2]) * 0.5
    U[3] = g[:, 2]
    # -> [CI, COT, 4, KH, 128]
    wr = np.ascontiguousarray(
        U.transpose(2, 0, 1, 3).reshape(CI, 4, KH, COT, 128)
        .transpose(0, 3, 1, 2, 4)).astype(bf16)
    bt_ = bias.reshape(COT, 128).T.astype(np.float32)     # [128, COT]
    b2 = np.ascontiguousarray(
        np.stack([bt_, -bt_], axis=-1))                   # [128, COT, 2]

    xp = np.pad(input, ((0, 0), (0, 0), (1, 1), (1, 1))).astype(bf16)
    xe = xp[:, :, :, 0::2]   # [N, CI, HP, 29]
    xo = xp[:, :, :, 1::2]
    planes = np.zeros((N, CI, HP, 4, XW), dtype=bf16)
    planes[:, :, :, 0, 0:28] = xe[:, :, :, 0:28]   # d0
    planes[:, :, :, 1, 0:28] = xo[:, :, :, 0:28]   # d1
    planes[:, :, :, 2, 0:28] = xe[:, :, :, 1:29]   # d2
    planes[:, :, :, 3, 0:28] = xo[:, :, :, 1:29]   # d3

    in_maps = []
    for c in range(N_CORES):
        xc = np.ascontiguousarray(
            planes[c * NP_CORE:(c + 1) * NP_CORE].transpose(1, 0, 2, 3, 4))
        in_maps.append({"xq": xc, "w": wr, "b2": b2})
    return in_maps


def kernel(input, weight, bias):
    input = np.asarray(input, dtype=np.float32)
    weight = np.asarray(weight, dtype=np.float32)
    bias = np.asarray(bias, dtype=np.float32)

    if "nc" not in _CACHE:
        _CACHE["nc"] = _build_program()
    nc = _CACHE["nc"]

    from concourse.bass_utils import run_bass_kernel_spmd

    in_maps = prep_in_maps(input, weight, bias)
    res = run_bass_kernel_spmd(nc, in_maps, core_ids=list(range(N_CORES)))

    out = np.empty((N, CO, H, W), dtype=np.float32)
    for c in range(N_CORES):
        y = np.asarray(res.results[c]["y"]).astype(np.float32)
        # [COT, 128, NP, 2, H, TX] -> [NP, COT, 128, H, TX, 2]
        y = y.transpose(2, 0, 1, 4, 5, 3).reshape(NP_CORE, CO, H, W)
        out[c * NP_CORE:(c + 1) * NP_CORE] = y
    return out


# revision 42
# speedup vs baseline: 1.0058x; 1.0058x over previous
"""Trainium2 Bass kernel for DenseConv2d via 1D Winograd F(2,3).

Conv2d: input (32,128,56,56) f32, weight (256,128,3,3) f32, bias (256,) f32,
stride 1, pad 1, dilation 1 -> output (32,256,56,56) f32.

Strategy: data-parallel over batch across 8 NeuronCores (4 images per core).
Per core, the conv is computed with Winograd F(2,3) along the x axis only:
for each x-tile tx (28 per row) the 4 transformed planes
  V0 = d0-d2, V1 = d1+d2, V2 = d2-d1, V3 = d1-d3   (dk = xpad[2tx+k])
are built on-chip by the vector engine from 4 pre-shifted even/odd input
planes (host-prepped so every operand is step-1 and 4B-aligned -> DVE 2x).
The y-axis stays direct: M_j = sum_ky U_j[ky]^T V_j[y+ky], so each of the
4 M-planes is a 3-matmul PSUM accumulation in bf16 (1 col/cycle + FWL).
This cuts PE columns 1.5x vs direct conv.  Outputs
  Y0 = M0+M1+M2+b  (even x),  Y1 = M1-M2-M3+b  (odd x)
drain from PSUM through three rotating engine paths (DVE fused
scalar_tensor_tensor; ACT strided copy + DVE adds; ACT copy + GPSIMD adds)
so no single engine bottlenecks.  Outputs ship as bf16; the host interleaves
even/odd columns and upcasts to f32 (layout-only).
"""

import sys

if "/opt/trn_rl_repo" not in sys.path:
    sys.path.insert(0, "/opt/trn_rl_repo")

import numpy as np

N_CORES = 8
N, CI, H, W = 32, 128, 56, 56
CO, KH, KW = 256, 3, 3
NP_CORE = N // N_CORES          # images per core
HP = H + 2                      # padded rows
TX = W // 2                     # x-tiles per row
XW = 32                         # padded plane row width (28 valid + pad)
COT = CO // 128                 # out-channel tiles of 128
RBW = 14                        # output rows per chunk
NCH = H // RBW                  # chunks per (img, cot)
FD = RBW * TX                   # matmul free dim (392)
N_WARMUP = 5                    # big PE warmup matmuls
N_TINY = 30                     # tiny warmup matmuls

_CACHE = {}


def _build_program():
    import concourse.mybir as mybir
    from concourse import bacc
    from concourse.tile import TileContext

    nc = bacc.Bacc(None, target_bir_lowering=False)

    bf16 = mybir.dt.bfloat16
    f32 = mybir.dt.float32
    ADD = mybir.AluOpType.add
    SUB = mybir.AluOpType.subtract
    COPY = mybir.ActivationFunctionType.Copy
    IDENT = mybir.ActivationFunctionType.Identity

    # xq planes: 0=xe[0:28] (d0), 1=xo[0:28] (d1), 2=xe[1:29] (d2),
    # 3=xo[1:29] (d3); all width-28 valid, padded to 32.
    x_d = nc.dram_tensor("xq", [CI, NP_CORE, 4, HP, XW], bf16,
                         kind="ExternalInput")
    w_d = nc.dram_tensor("w", [CI, COT, 4, KH, 128], bf16,
                         kind="ExternalInput")
    # [:, cot, 0] = +bias, [:, cot, 1] = -bias
    b_d = nc.dram_tensor("b2", [128, COT, 2], f32, kind="ExternalInput")
    # y layout: [cot, co_p, img, b(even/odd), y, tx]
    y_d = nc.dram_tensor("y", [COT, 128, NP_CORE, 2, H, TX], bf16,
                         kind="ExternalOutput")

    with TileContext(nc) as tc:
        with (
            tc.tile_pool(name="xin", bufs=1) as xpool,
            tc.tile_pool(name="vpool", bufs=1) as vpool,
            tc.tile_pool(name="wpool", bufs=1) as wpool,
            tc.tile_pool(name="bpool", bufs=1) as bpool,
            tc.tile_pool(name="cpool", bufs=3) as cpool,
            tc.tile_pool(name="tpool", bufs=2) as tpool,
            tc.tile_pool(name="ypool", bufs=3) as ypool,
            tc.tile_pool(name="psum", bufs=2, space="PSUM") as ppool,
        ):
            # --- PE warmup on scratch data (lifts HAM clock gate) ---
            scratch = xpool.tile([CI, FD], bf16, tag="scratch")
            nc.vector.memset(scratch, 0.0)
            wups = ppool.tile([128, 4, 512], f32, tag="m")
            for _ in range(N_WARMUP):
                nc.tensor.matmul(wups[:, 0, 0:FD], scratch[:, 0:128],
                                 scratch, start=True, stop=True)
            for _ in range(N_TINY):
                nc.tensor.matmul(wups[:, 0, 0:64], scratch[:, 0:128],
                                 scratch[:, 0:64], start=True, stop=True)

            # --- DMAs: weights + bias on sync, input planes on scalar ---
            # Input DMAs all ride the scalar ring, img0 first in 4 row-chunks
            # (same-ring transfers serialize, so img0 is prioritized and its
            # first chunk lands fast); weights + bias ride sync.  All y-out
            # DMAs go on sync LATER so they never block the scalar ring's
            # PSUM-drain copies.
            xt = [xpool.tile([CI, 4, HP, XW], bf16, tag=f"x{img}",
                             name=f"x{img}")
                  for img in range(NP_CORE)]
            X0CH = [(0, 16), (16, 30), (30, 44), (44, HP)]
            for r0, r1 in X0CH:
                nc.scalar.dma_start(out=xt[0][:, :, r0:r1, :],
                                    in_=x_d[:, 0, :, r0:r1, :])
            wt = []
            for cot in range(COT):
                wtile = wpool.tile([CI, 4, KH, 128], bf16, tag=f"w{cot}")
                nc.sync.dma_start(out=wtile, in_=w_d[:, cot, :, :, :])
                wt.append(wtile)
            bt = bpool.tile([128, COT, 2], f32)
            nc.sync.dma_start(out=bt, in_=b_d[:, :, :])
            # x1/x2/x3 queue behind x0 on the scalar ring, serialized in
            # image order.  Total early HBM bandwidth is the binding
            # constraint: spreading these across extra rings floods the
            # fabric and starves x0's critical first chunk (measured).
            for img in range(1, NP_CORE):
                nc.scalar.dma_start(out=xt[img], in_=x_d[:, img, :, :, :])

            # only 2 V tiles live at once (current image + prefetched next)
            vt = [vpool.tile([CI, 4, HP, XW], bf16, tag=f"v{img % 2}",
                             name=f"v{img}")
                  for img in range(NP_CORE)]

            def emit_v(img, r0, r1, eng=None):
                eng = eng or nc.vector
                x_ = xt[img]
                v_ = vt[img]
                p = [x_[:, k, r0:r1, 0:28] for k in range(4)]
                o = [v_[:, j, r0:r1, 0:28] for j in range(4)]
                eng.tensor_tensor(out=o[0], in0=p[0], in1=p[2], op=SUB)
                eng.tensor_tensor(out=o[1], in0=p[1], in1=p[2], op=ADD)
                eng.tensor_tensor(out=o[2], in0=p[2], in1=p[1], op=SUB)
                eng.tensor_tensor(out=o[3], in0=p[1], in1=p[3], op=SUB)

            # Per (img, cot): 4 chunks processed as 2 pairs.  Each chunk's
            # 4 M-planes drain via ONE strided ACT copy (frees the PSUM
            # buffer quickly); the Winograd output adds then run over the
            # PAIR (FD=784, amortizing fixed op costs), all on DVE with the
            # bias fused into the first layer:
            #   ye = ((c0 + b) + c1) + c2   = M0+M1+M2+b
            #   yo = ((c1 + b) - c2) - c3   = M1-M2-M3+b
            def emit_chunk_mms(img, cot, ch, c2):
                y0 = ch * RBW
                mt = ppool.tile([128, 4, 512], f32, tag="m")
                for j in range(4):
                    for ky in range(KH):
                        rhs = vt[img][:, j, y0 + ky:y0 + ky + RBW, 0:28]
                        nc.tensor.matmul(
                            mt[:, j, 0:FD], wt[cot][:, j, ky, :], rhs,
                            start=(ky == 0), stop=(ky == KH - 1),
                        )
                nc.scalar.activation(out=c2[:, :, ch % 2, :],
                                     in_=mt[:, :, 0:FD], func=COPY)

            def emit_pair_adds(img, cot, pair, c2, yt):
                bpos = bt[:, cot, 0:1]
                cc = [c2[:, k, :, :] for k in range(4)]   # [128, 2, FD] flat
                ye = yt[:, 0, 2 * pair:2 * pair + 2, :]
                yo = yt[:, 1, 2 * pair:2 * pair + 2, :]
                t0 = tpool.tile([128, 2, FD], bf16, tag="t0")
                nc.vector.scalar_tensor_tensor(
                    out=t0, in0=cc[0], scalar=bpos, in1=cc[1],
                    op0=ADD, op1=ADD)
                t1 = tpool.tile([128, 2, FD], bf16, tag="t1")
                nc.vector.scalar_tensor_tensor(
                    out=t1, in0=cc[1], scalar=bpos, in1=cc[2],
                    op0=ADD, op1=SUB)
                nc.vector.tensor_tensor(out=ye, in0=t0, in1=cc[2], op=ADD)
                nc.vector.tensor_tensor(out=yo, in0=t1, in1=cc[3], op=SUB)

            def emit_compute(img, cot, yt):
                for pair in range(2):
                    # c2 packs both chunks of the pair: [plane, set, FD]
                    c2 = cpool.tile([128, 4, 2, FD], bf16, tag="c")
                    emit_chunk_mms(img, cot, 2 * pair, c2)
                    emit_chunk_mms(img, cot, 2 * pair + 1, c2)
                    emit_pair_adds(img, cot, pair, c2, yt)
                    # ship this half-image's rows while the next pair drains
                    nc.sync.dma_start(
                        out=y_d[cot, :, img, :,
                                pair * 2 * RBW:(pair + 1) * 2 * RBW, :],
                        in_=yt[:, :, 2 * pair:2 * pair + 2, :])

            def emit_compute_tail(img, cot, yt):
                # Final (img, cot): chunks 2+3 drain as singletons with M0/M3
                # read straight from PSUM (one PSUM operand per op is legal)
                # so the post-last-matmul chain is ~2us shorter: only planes
                # 1+2 are copied, and each chunk ships immediately.
                bpos = bt[:, cot, 0:1]
                c2 = cpool.tile([128, 4, 2, FD], bf16, tag="c")
                emit_chunk_mms(img, cot, 0, c2)
                emit_chunk_mms(img, cot, 1, c2)
                emit_pair_adds(img, cot, 0, c2, yt)
                nc.sync.dma_start(out=y_d[cot, :, img, :, 0:2 * RBW, :],
                                  in_=yt[:, :, 0:2, :])
                for ch in (2, 3):
                    y0 = ch * RBW
                    mt = ppool.tile([128, 4, 512], f32, tag="m")
                    for j in range(4):
                        for ky in range(KH):
                            rhs = vt[img][:, j, y0 + ky:y0 + ky + RBW, 0:28]
                            nc.tensor.matmul(
                                mt[:, j, 0:FD], wt[cot][:, j, ky, :], rhs,
                                start=(ky == 0), stop=(ky == KH - 1),
                            )
                    ct = cpool.tile([128, 4, 2, FD], bf16, tag="c")
                    nc.scalar.activation(out=ct[:, 1:3, 0, :],
                                         in_=mt[:, 1:3, 0:FD], func=COPY)
                    c1 = ct[:, 1, 0, :]
                    c2s = ct[:, 2, 0, :]
                    ye = yt[:, 0, ch, :]
                    yo = yt[:, 1, ch, :]
                    t0 = tpool.tile([128, 2, FD], bf16, tag="t0")
                    nc.vector.scalar_tensor_tensor(
                        out=t0[:, 0, :], in0=mt[:, 0, 0:FD], scalar=bpos,
                        in1=c1, op0=ADD, op1=ADD)
                    nc.vector.tensor_tensor(out=ye, in0=t0[:, 0, :],
                                            in1=c2s, op=ADD)
                    t1 = tpool.tile([128, 2, FD], bf16, tag="t1")
                    nc.vector.scalar_tensor_tensor(
                        out=t1[:, 0, :], in0=c1, scalar=bpos, in1=c2s,
                        op0=ADD, op1=SUB)
                    nc.vector.tensor_tensor(out=yo, in0=t1[:, 0, :],
                                            in1=mt[:, 3, 0:FD], op=SUB)
                    q = nc.scalar if ch == 3 else nc.sync
                    q.dma_start(
                        out=y_d[cot, :, img, :, ch * RBW:(ch + 1) * RBW, :],
                        in_=yt[:, :, ch:ch + 1, :])

            def emit_compute_last(img, cot, yt):
                # final (img, cot): per-chunk drains + small DMAs so the
                # post-last-matmul tail is as short as possible; the last
                # DMA rides the scalar ring (idle once its copies are done).
                bpos = bt[:, cot, 0:1]
                for ch in range(NCH):
                    c2 = cpool.tile([128, 4, 2, FD], bf16, tag="c")
                    emit_chunk_mms(img, cot, ch, c2)
                    cc = [c2[:, k, ch % 2, :] for k in range(4)]
                    ye = yt[:, 0, ch, :]
                    yo = yt[:, 1, ch, :]
                    t0 = tpool.tile([128, 2, FD], bf16, tag="t0")
                    nc.vector.scalar_tensor_tensor(
                        out=t0[:, 0, :], in0=cc[0], scalar=bpos, in1=cc[1],
                        op0=ADD, op1=ADD)
                    t1 = tpool.tile([128, 2, FD], bf16, tag="t1")
                    nc.vector.scalar_tensor_tensor(
                        out=t1[:, 0, :], in0=cc[1], scalar=bpos, in1=cc[2],
                        op0=ADD, op1=SUB)
                    nc.vector.tensor_tensor(out=ye, in0=t0[:, 0, :],
                                            in1=cc[2], op=ADD)
                    nc.vector.tensor_tensor(out=yo, in0=t1[:, 0, :],
                                            in1=cc[3], op=SUB)
                    q = nc.scalar if ch == NCH - 1 else nc.sync
                    q.dma_start(
                        out=y_d[cot, :, img, :, ch * RBW:(ch + 1) * RBW, :],
                        in_=yt[:, :, ch:ch + 1, :])

            for r0, r1 in X0CH:
                emit_v(0, r0, r1)
            for img in range(NP_CORE):
                for cot in range(COT):
                    yt = ypool.tile([128, 2, NCH, FD], bf16, tag="y")
                    # prefetch next image's V transform between cots
                    if cot == 1 and img + 1 < NP_CORE:
                        emit_v(img + 1, 0, HP)
                    if img == NP_CORE - 1 and cot == COT - 1:
                        emit_compute_tail(img, cot, yt)
                    else:
                        emit_compute(img, cot, yt)

    nc.compile()
    return nc


def prep_in_maps(input, weight, bias):
    """Host-side layout prep -> one in_map per core."""
    import ml_dtypes

    bf16 = ml_dtypes.bfloat16

    # Winograd weight transform (tiny): U_j[ky][ci, co]
    g = weight.transpose(2, 3, 1, 0).astype(np.float32)  # [kh, kw, ci, co]
    U = np.empty((4, KH, CI, CO), dtype=np.float32)
    U[0] = g[:, 0]
    U[1] = (g[:, 0] + g[:, 1] + g[:, 2]) * 0.5
    U[2] = (g[:, 0] - g[:, 1] + g[:, 2]) * 0.5
    U[3] = g[:, 2]
    # -> [CI, COT, 4, KH, 128]
    wr = np.ascontiguousarray(
        U.transpose(2, 0, 1, 3).reshape(CI, 4, KH, COT, 128)
        .transpose(0, 3, 1, 2, 4)).astype(bf16)
    bt_ = bias.reshape(COT, 128).T.astype(np.float32)     # [128, COT]
    b2 = np.ascontiguousarray(
        np.stack([bt_, -bt_], axis=-1))                   # [128, COT, 2]

    xp = np.pad(input, ((0, 0), (0, 0), (1, 1), (1, 1))).astype(bf16)
    xe = xp[:, :, :, 0::2]   # [N, CI, HP, 29]
    xo = xp[:, :, :, 1::2]
    planes = np.zeros((N, CI, 4, HP, XW), dtype=bf16)
    planes[:, :, 0, :, 0:28] = xe[:, :, :, 0:28]   # d0
    planes[:, :, 1, :, 0:28] = xo[:, :, :, 0:28]   # d1
    planes[:, :, 2, :, 0:28] = xe[:, :, :, 1:29]   # d2
    planes[:, :, 3, :, 0:28] = xo[:, :, :, 1:29]   # d3

    in_maps = []
    for c in range(N_CORES):
        xc = np.ascontiguousarray(
            planes[c * NP_CORE:(c + 1) * NP_CORE].transpose(1, 0, 2, 3, 4))
        in_maps.append({"xq": xc, "w": wr, "b2": b2})
    return in_maps


def kernel(input, weight, bias):
    input = np.asarray(input, dtype=np.float32)
    weight = np.asarray(weight, dtype=np.float32)
    bias = np.asarray(bias, dtype=np.float32)

    if "nc" not in _CACHE:
        _CACHE["nc"] = _build_program()
    nc = _CACHE["nc"]

    from concourse.bass_utils import run_bass_kernel_spmd

    in_maps = prep_in_maps(input, weight, bias)
    res = run_bass_kernel_spmd(nc, in_maps, core_ids=list(range(N_CORES)))

    out = np.empty((N, CO, H, W), dtype=np.float32)
    for c in range(N_CORES):
        y = np.asarray(res.results[c]["y"]).astype(np.float32)
        # [COT, 128, NP, 2, H, TX] -> [NP, COT, 128, H, TX, 2]
        y = y.transpose(2, 0, 1, 4, 5, 3).reshape(NP_CORE, CO, H, W)
        out[c * NP_CORE:(c + 1) * NP_CORE] = y
    return out
